# revision 1
# baseline (speedup 1.0000x reference)
"""3-layer GCN (DropGNN inference) on 8 Trainium2 NeuronCores.

Sharding: nodes row-sharded across 8 cores (6272 rows each, padded to 50176);
edges partitioned by destination node; weight matrices replicated; per-layer
transformed features M = H @ W are AllGathered so every core can gather
arbitrary source rows (equivalent to halo-exchanging gathered src features).

Per core, per layer:
  dense:  for each of 49 node tiles, PE computes M tile = (H^T tile)^T @ W
          (H kept feature-major in SBUF so no transposes are ever needed),
          ACT copies PSUM->SBUF, one DMA writes the [6272, fo] shard to DRAM.
  AG:     AllGather shards -> full M [50176, fo] in DRAM.
  edge:   for each dst tile: dma_gather pulls the tile's (padded) edge source
          rows (512B each) from M; DVE builds per-128-edge-block one-hot
          matrices onehot[e, d] = (iota[d] == dst_local[e]) * norm[e] in a
          single fused tensor_scalar op; PE accumulates
          PSUM[f, d] += msgs[e, f]^T @ onehot[e, d]
          over the tile's blocks -- segment_sum as dense matmuls. Epilogue:
          ACT relu(PSUM + b) -> next layer's feature-major H tile.
Final layer: PE-transpose logits to node-major, fused exp+accum log-softmax,
one DMA out. Host gathers the 8 shards and slices to 50000 rows.
"""

import os
import sys

import numpy as np

for _p in ("/opt/trn_rl_repo", "/opt/trn_rl_repo/concourse"):
    if _p not in sys.path:
        sys.path.insert(0, _p)

N = 50000
E = 625000
F_IN = 128
HID = 128
N_CLASSES = 64
NCORES = 8
TPC = 49                      # 128-node tiles per core
SH = TPC * 128                # 6272 nodes per core
NPAD = NCORES * SH            # 50176
HALF = NPAD // 2              # 25088 (< int16 max, gather index limit)
FO = (HID, HID, N_CLASSES)    # per-layer output widths

LAST_RESULTS = None           # BassKernelResults of the most recent run


def _preprocess(x, edge_index):
    """Sort/partition edges by destination, build per-core gather indices and
    per-block (dst_local, norm) metadata with fixed per-tile block counts."""
    ei = np.asarray(edge_index).astype(np.int64)
    loop = np.arange(N, dtype=np.int64)
    src = np.concatenate([ei[0], loop])
    dst = np.concatenate([ei[1], loop])

    deg = np.bincount(dst, minlength=NPAD).astype(np.float32)
    dinv = np.zeros(NPAD, np.float32)
    nz = deg > 0
    dinv[nz] = (np.float32(1.0) / np.sqrt(deg[nz])).astype(np.float32)
    norm = (dinv[src] * dinv[dst]).astype(np.float32)

    order = np.argsort(dst, kind="stable")
    src_s, dst_s, nrm_s = src[order], dst[order], norm[order]
    # global tile g = c*TPC + t covers dst rows [g*128, (g+1)*128)
    bounds = np.searchsorted(dst_s, np.arange(0, NPAD + 1, 128))

    ntiles = NCORES * TPC
    groups = []  # per (g, half): (src_rel, dst_local, norm)
    nA = np.zeros(ntiles, np.int64)
    nB = np.zeros(ntiles, np.int64)
    for g in range(ntiles):
        sl = slice(bounds[g], bounds[g + 1])
        es, el, en = src_s[sl], (dst_s[sl] - g * 128), nrm_s[sl]
        mA = es < HALF
        a = (es[mA], el[mA], en[mA])
        b = (es[~mA] - HALF, el[~mA], en[~mA])
        nA[g], nB[g] = len(a[0]), len(b[0])
        groups.append((a, b))

    nA2 = nA.reshape(NCORES, TPC)
    nB2 = nB.reshape(NCORES, TPC)
    kA = [int(x) for x in np.ceil(nA2.max(axis=0) / 128).astype(np.int64)]
    kB = [int(x) for x in np.ceil(nB2.max(axis=0) / 128).astype(np.int64)]
    for t in range(TPC):
        if kA[t] + kB[t] == 0:
            kA[t] = 1  # keep at least one (all-pad) block so PSUM gets started
    NB = sum(kA) + sum(kB)

    idx_all, mdst_all, mnrm_all = [], [], []
    for c in range(NCORES):
        idx16 = np.zeros((16, NB * 8), np.int16)
        mdst = np.full((128, NB), -1.0, np.float32)
        mnrm = np.zeros((128, NB), np.float32)
        bcol = 0
        for t in range(TPC):
            a, b = groups[c * TPC + t]
            for (es, el, en), k in ((a, kA[t]), (b, kB[t])):
                if k == 0:
                    continue
                n = len(es)
                if n:
                    s = np.arange(n)
                    mdst[s % 128, bcol + s // 128] = el.astype(np.float32)
                    mnrm[s % 128, bcol + s // 128] = en
                j = np.arange(k * 128)
                vals = np.zeros(k * 128, np.int16)
                vals[:n] = es.astype(np.int16)
                idx16[j % 16, bcol * 8 + j // 16] = vals
                bcol += k
        assert bcol == NB
        idx_all.append(np.tile(idx16, (8, 1)))
        mdst_all.append(mdst)
        mnrm_all.append(mnrm)

    return kA, kB, NB, idx_all, mdst_all, mnrm_all


def _build(kA, kB, NB):
    import concourse.bacc as bacc
    import concourse.mybir as mybir
    import concourse.tile as tile
    from concourse import masks

    f32 = mybir.dt.float32
    i16 = mybir.dt.int16
    Alu = mybir.AluOpType
    Act = mybir.ActivationFunctionType

    nc = bacc.Bacc(None, target_bir_lowering=False, num_devices=NCORES)

    xT_d = nc.dram_tensor("xT", [128, SH], f32, kind="ExternalInput")
    w1_d = nc.dram_tensor("w1", [128, HID], f32, kind="ExternalInput")
    w2_d = nc.dram_tensor("w2", [128, HID], f32, kind="ExternalInput")
    wf_d = nc.dram_tensor("wf", [128, N_CLASSES], f32, kind="ExternalInput")
    bias_d = nc.dram_tensor("bias", [128, 3], f32, kind="ExternalInput")
    iota_d = nc.dram_tensor("iota", [128, 128], f32, kind="ExternalInput")
    mdst_d = nc.dram_tensor("mdst", [128, NB], f32, kind="ExternalInput")
    mnrm_d = nc.dram_tensor("mnrm", [128, NB], f32, kind="ExternalInput")
    idx_d = nc.dram_tensor("idx", [128, NB * 8], i16, kind="ExternalInput")
    out_d = nc.dram_tensor("logits", [SH, N_CLASSES], f32, kind="ExternalOutput")

    rg = [list(range(NCORES))]

    with tile.TileContext(nc, num_cores=NCORES) as tc:
        with (
            tc.tile_pool(name="const", bufs=1) as cp,
            tc.tile_pool(name="hbuf", bufs=1) as hp,
            tc.tile_pool(name="stage", bufs=1) as sp,
            tc.tile_pool(name="msgs", bufs=2) as mp,
            tc.tile_pool(name="oh", bufs=4) as op_,
            tc.tile_pool(name="lt", bufs=4) as lp,
            tc.tile_pool(name="small", bufs=4) as zp,
            tc.tile_pool(name="pm", bufs=2, space="PSUM") as pmp,
            tc.tile_pool(name="ph", bufs=2, space="PSUM") as php,
            tc.tile_pool(name="pt", bufs=2, space="PSUM") as ptp,
            tc.tile_pool(name="dram", bufs=1, space="DRAM") as dp,
        ):
            w1 = cp.tile([128, HID], f32)
            w2 = cp.tile([128, HID], f32)
            wf = cp.tile([128, N_CLASSES], f32)
            bias = cp.tile([128, 3], f32)
            iota = cp.tile([128, 128], f32)
            mdst = cp.tile([128, NB], f32)
            mnrm = cp.tile([128, NB], f32)
            idxs = cp.tile([128, NB * 8], i16)
            ident = cp.tile([128, 128], f32)

            h0 = hp.tile([128, SH], f32, tag="h0")   # layer-l input, feature-major
            h1 = hp.tile([128, SH], f32, tag="h1")
            logits = sp.tile([128, TPC * N_CLASSES], f32, tag="lg")
            xs = sp.tile([128, TPC * N_CLASSES], f32, tag="xs")
            outn = sp.tile([128, TPC * N_CLASSES], f32, tag="on")
            sums = sp.tile([128, TPC], f32, tag="sm")
            lsum = sp.tile([128, TPC], f32, tag="ls")

            nc.sync.dma_start(w1[:], w1_d[:])
            nc.sync.dma_start(w2[:], w2_d[:])
            nc.sync.dma_start(wf[:], wf_d[:])
            nc.sync.dma_start(bias[:], bias_d[:])
            nc.sync.dma_start(iota[:], iota_d[:])
            nc.sync.dma_start(mdst[:], mdst_d[:])
            nc.sync.dma_start(mnrm[:], mnrm_d[:])
            nc.sync.dma_start(idxs[:], idx_d[:])
            nc.sync.dma_start(h0[:], xT_d[:])
            masks.make_identity(nc, ident[:])

            # per-tile block column offsets (shared host/device bookkeeping)
            boff = []
            acc = 0
            for t in range(TPC):
                boff.append(acc)
                acc += kA[t] + kB[t]

            hcur = h0
            hnxt = h1
            weights = (w1, w2, wf)
            for L in range(3):
                fo = FO[L]
                W = weights[L]

                # ---- dense phase: M shard = (H^T)^T @ W, node-major ----
                msh = dp.tile([SH, fo], f32, tag=f"msh{L}")
                mg = dp.tile([NPAD, fo], f32, tag=f"mg{L}")
                mstage = sp.tile([128, TPC * fo], f32, tag=f"mst{L % 2}")
                for t in range(TPC):
                    pm = pmp.tile([128, fo], f32, tag="pm")
                    nc.tensor.matmul(
                        pm[:], hcur[:, t * 128:(t + 1) * 128], W[:],
                        start=True, stop=True,
                    )
                    nc.scalar.activation(
                        mstage[:, t * fo:(t + 1) * fo], pm[:], Act.Copy
                    )
                nc.sync.dma_start(
                    msh[:].rearrange("(t p) f -> p t f", p=128),
                    mstage[:].rearrange("p (t f) -> p t f", f=fo),
                )

                # ---- AllGather M shards -> full M ----
                nc.gpsimd.collective_compute(
                    "AllGather",
                    mybir.AluOpType.bypass,
                    replica_groups=rg,
                    ins=[msh[:].opt()],
                    outs=[mg[:].opt()],
                )

                # ---- edge phase ----
                for t in range(TPC):
                    ka, kb = kA[t], kB[t]
                    kt = ka + kb
                    c0 = boff[t]
                    msgs = mp.tile([128, kt, fo], f32, tag="msgs")
                    if ka:
                        nc.gpsimd.dma_gather(
                            msgs[:, 0:ka, :],
                            mg[0:HALF, :],
                            idxs[:, c0 * 8:(c0 + ka) * 8],
                            ka * 128, ka * 128, fo,
                        )
                    if kb:
                        nc.gpsimd.dma_gather(
                            msgs[:, ka:kt, :],
                            mg[HALF:NPAD, :],
                            idxs[:, (c0 + ka) * 8:(c0 + kt) * 8],
                            kb * 128, kb * 128, fo,
                        )
                    ph = php.tile([fo, 128], f32, tag="ph")
                    for b in range(kt):
                        g = c0 + b
                        oh = op_.tile([128, 128], f32, tag="oh")
                        nc.vector.tensor_scalar(
                            oh[:], iota[:],
                            mdst[:, g:g + 1], mnrm[:, g:g + 1],
                            Alu.is_equal, Alu.mult,
                        )
                        nc.tensor.matmul(
                            ph[:], msgs[:, b, :], oh[:],
                            start=(b == 0), stop=(b == kt - 1),
                        )
                    if L < 2:
                        nc.scalar.activation(
                            hnxt[:, t * 128:(t + 1) * 128], ph[:],
                            Act.Relu, bias=bias[:, L:L + 1],
                        )
                    else:
                        lt = lp.tile([N_CLASSES, 128], f32, tag="lt")
                        nc.scalar.activation(
                            lt[:], ph[:], Act.Identity,
                            bias=bias[:N_CLASSES, 2:3],
                        )
                        pt = ptp.tile([128, N_CLASSES], f32, tag="pt")
                        nc.tensor.transpose(
                            pt[:], lt[:], ident[:N_CLASSES, :N_CLASSES]
                        )
                        nc.vector.tensor_copy(
                            logits[:, t * N_CLASSES:(t + 1) * N_CLASSES], pt[:]
                        )
                hcur, hnxt = hnxt, hcur

            # ---- log-softmax over the 64 classes (free dim, node-major) ----
            lg3 = logits[:].rearrange("p (t c) -> p t c", c=N_CLASSES)
            xs3 = xs[:].rearrange("p (t c) -> p t c", c=N_CLASSES)
            on3 = outn[:].rearrange("p (t c) -> p t c", c=N_CLASSES)
            for t in range(TPC):
                mx = zp.tile([128, 1], f32, tag="mx")
                nc.vector.tensor_reduce(
                    mx[:], lg3[:, t, :], mybir.AxisListType.X, Alu.max
                )
                nc.vector.tensor_scalar(
                    xs3[:, t, :], lg3[:, t, :], mx[:], None, Alu.subtract
                )
                junk = zp.tile([128, N_CLASSES], f32, tag="jk")
                nc.scalar.activation(
                    junk[:], xs3[:, t, :], Act.Exp,
                    accum_out=sums[:, t:t + 1],
                )
            nc.scalar.activation(lsum[:], sums[:], Act.Ln)
            for t in range(TPC):
                nc.vector.tensor_scalar(
                    on3[:, t, :], xs3[:, t, :], lsum[:, t:t + 1], None,
                    Alu.subtract,
                )
            nc.sync.dma_start(
                out_d[:].rearrange("(t p) c -> p t c", p=128),
                on3,
            )

    nc.compile()
    return nc


def kernel(x, edge_index, W1, b1, W2, b2, Wf, bf):
    global LAST_RESULTS
    from concourse.bass_utils import run_bass_kernel_spmd

    x = np.asarray(x, dtype=np.float32)
    W1 = np.asarray(W1, dtype=np.float32)
    b1 = np.asarray(b1, dtype=np.float32)
    W2 = np.asarray(W2, dtype=np.float32)
    b2 = np.asarray(b2, dtype=np.float32)
    Wf = np.asarray(Wf, dtype=np.float32)
    bf = np.asarray(bf, dtype=np.float32)

    kA, kB, NB, idx_all, mdst_all, mnrm_all = _preprocess(x, edge_index)
    nc = _build(kA, kB, NB)

    xpad = np.zeros((NPAD, F_IN), np.float32)
    xpad[:N] = x
    bias = np.zeros((128, 3), np.float32)
    bias[:, 0] = b1
    bias[:, 1] = b2
    bias[:N_CLASSES, 2] = bf
    iota = np.tile(np.arange(128, dtype=np.float32), (128, 1))

    in_maps = []
    for c in range(NCORES):
        in_maps.append({
            "xT": np.ascontiguousarray(xpad[c * SH:(c + 1) * SH].T),
            "w1": W1, "w2": W2, "wf": Wf,
            "bias": bias, "iota": iota,
            "mdst": mdst_all[c], "mnrm": mnrm_all[c], "idx": idx_all[c],
        })

    res = run_bass_kernel_spmd(
        nc, in_maps, core_ids=list(range(NCORES)),
        trace=bool(os.environ.get("BASS_TRACE")),
    )
    LAST_RESULTS = res
    if res.exec_time_ns is not None:
        print(f"HW exec time: {res.exec_time_ns} ns")

    out = np.concatenate([r["logits"] for r in res.results], axis=0)
    return out[:N].astype(np.float32)


# revision 3
# speedup vs baseline: 1.7135x; 1.7135x over previous
"""3-layer GCN (DropGNN inference) on 8 Trainium2 NeuronCores.

Sharding: nodes row-sharded across 8 cores (6272 rows each, padded to 50176);
edges partitioned by destination node; weight matrices replicated; per-layer
transformed features M = H @ W are AllGathered so every core can gather
arbitrary source rows (equivalent to halo-exchanging gathered src features).

Per core, per layer:
  dense:  for each of 49 node tiles, PE computes M tile = (H^T tile)^T @ W
          (H kept feature-major in SBUF so no transposes are ever needed),
          ACT copies PSUM->SBUF, one DMA writes the [6272, fo] shard to DRAM.
  AG:     AllGather shards -> full M [50176, fo] in DRAM.
  edge:   for each dst tile: dma_gather pulls the tile's (padded) edge source
          rows (512B each) from M; DVE builds per-128-edge-block one-hot
          matrices onehot[e, d] = (iota[d] == dst_local[e]) * norm[e] in a
          single fused tensor_scalar op; PE accumulates
          PSUM[f, d] += msgs[e, f]^T @ onehot[e, d]
          over the tile's blocks -- segment_sum as dense matmuls. Epilogue:
          ACT relu(PSUM + b) -> next layer's feature-major H tile.
Final layer: PE-transpose logits to node-major, fused exp+accum log-softmax,
one DMA out. Host gathers the 8 shards and slices to 50000 rows.
"""

import os
import sys

import numpy as np

for _p in ("/opt/trn_rl_repo", "/opt/trn_rl_repo/concourse"):
    if _p not in sys.path:
        sys.path.insert(0, _p)

N = 50000
E = 625000
F_IN = 128
HID = 128
N_CLASSES = 64
NCORES = 8
TPC = 49                      # 128-node tiles per core
SH = TPC * 128                # 6272 nodes per core
NPAD = NCORES * SH            # 50176
HALF = NPAD // 2              # 25088 (< int16 max, gather index limit)
FO = (HID, HID, N_CLASSES)    # per-layer output widths

LAST_RESULTS = None           # BassKernelResults of the most recent run


def _preprocess(x, edge_index):
    """Sort/partition edges by destination, build per-core gather indices and
    per-block (dst_local, norm) metadata with fixed per-tile block counts."""
    ei = np.asarray(edge_index).astype(np.int64)
    loop = np.arange(N, dtype=np.int64)
    src = np.concatenate([ei[0], loop])
    dst = np.concatenate([ei[1], loop])

    deg = np.bincount(dst, minlength=NPAD).astype(np.float32)
    dinv = np.zeros(NPAD, np.float32)
    nz = deg > 0
    dinv[nz] = (np.float32(1.0) / np.sqrt(deg[nz])).astype(np.float32)
    norm = (dinv[src] * dinv[dst]).astype(np.float32)

    order = np.argsort(dst, kind="stable")
    src_s, dst_s, nrm_s = src[order], dst[order], norm[order]
    # global tile g = c*TPC + t covers dst rows [g*128, (g+1)*128)
    bounds = np.searchsorted(dst_s, np.arange(0, NPAD + 1, 128))

    ntiles = NCORES * TPC
    groups = []  # per (g, half): (src_rel, dst_local, norm)
    nA = np.zeros(ntiles, np.int64)
    nB = np.zeros(ntiles, np.int64)
    for g in range(ntiles):
        sl = slice(bounds[g], bounds[g + 1])
        es, el, en = src_s[sl], (dst_s[sl] - g * 128), nrm_s[sl]
        mA = es < HALF
        a = (es[mA], el[mA], en[mA])
        b = (es[~mA] - HALF, el[~mA], en[~mA])
        nA[g], nB[g] = len(a[0]), len(b[0])
        groups.append((a, b))

    nA2 = nA.reshape(NCORES, TPC)
    nB2 = nB.reshape(NCORES, TPC)
    kA = [int(x) for x in np.ceil(nA2.max(axis=0) / 128).astype(np.int64)]
    kB = [int(x) for x in np.ceil(nB2.max(axis=0) / 128).astype(np.int64)]
    for t in range(TPC):
        if kA[t] + kB[t] == 0:
            kA[t] = 1  # keep at least one (all-pad) block so PSUM gets started
    NB = sum(kA) + sum(kB)

    idx_all, mdst_all, mnrm_all = [], [], []
    for c in range(NCORES):
        idx16 = np.zeros((16, NB * 8), np.int16)
        mdst = np.full((128, NB), -1.0, np.float32)
        mnrm = np.zeros((128, NB), np.float32)
        bcol = 0
        for t in range(TPC):
            a, b = groups[c * TPC + t]
            for (es, el, en), k in ((a, kA[t]), (b, kB[t])):
                if k == 0:
                    continue
                n = len(es)
                if n:
                    s = np.arange(n)
                    mdst[s % 128, bcol + s // 128] = el.astype(np.float32)
                    mnrm[s % 128, bcol + s // 128] = en
                j = np.arange(k * 128)
                vals = np.zeros(k * 128, np.int16)
                vals[:n] = es.astype(np.int16)
                idx16[j % 16, bcol * 8 + j // 16] = vals
                bcol += k
        assert bcol == NB
        idx_all.append(np.tile(idx16, (8, 1)))
        mdst_all.append(mdst)
        mnrm_all.append(mnrm)

    return kA, kB, NB, idx_all, mdst_all, mnrm_all


def _build(kA, kB, NB):
    import concourse.bacc as bacc
    import concourse.mybir as mybir
    import concourse.tile as tile
    from concourse import masks

    f32 = mybir.dt.float32
    i16 = mybir.dt.int16
    Alu = mybir.AluOpType
    Act = mybir.ActivationFunctionType

    nc = bacc.Bacc(
        None, target_bir_lowering=False, num_devices=NCORES,
        num_swdge_queues=4,
    )

    xT_d = nc.dram_tensor("xT", [128, SH], f32, kind="ExternalInput")
    w1_d = nc.dram_tensor("w1", [128, HID], f32, kind="ExternalInput")
    w2_d = nc.dram_tensor("w2", [128, HID], f32, kind="ExternalInput")
    wf_d = nc.dram_tensor("wf", [128, N_CLASSES], f32, kind="ExternalInput")
    bias_d = nc.dram_tensor("bias", [128, 3], f32, kind="ExternalInput")
    iota_d = nc.dram_tensor("iota", [128, 128], f32, kind="ExternalInput")
    mdst_d = nc.dram_tensor("mdst", [128, NB], f32, kind="ExternalInput")
    mnrm_d = nc.dram_tensor("mnrm", [128, NB], f32, kind="ExternalInput")
    idx_d = nc.dram_tensor("idx", [128, NB * 8], i16, kind="ExternalInput")
    out_d = nc.dram_tensor("logits", [SH, N_CLASSES], f32, kind="ExternalOutput")

    rg = [list(range(NCORES))]

    with tile.TileContext(nc, num_cores=NCORES) as tc:
        with (
            tc.tile_pool(name="const", bufs=1) as cp,
            tc.tile_pool(name="hbuf", bufs=1) as hp,
            tc.tile_pool(name="stage", bufs=1) as sp,
            tc.tile_pool(name="msgs", bufs=2) as mp,
            tc.tile_pool(name="oh", bufs=4) as op_,
            tc.tile_pool(name="lt", bufs=4) as lp,
            tc.tile_pool(name="small", bufs=4) as zp,
            tc.tile_pool(name="pm", bufs=2, space="PSUM") as pmp,
            tc.tile_pool(name="ph", bufs=2, space="PSUM") as php,
            tc.tile_pool(name="pt", bufs=2, space="PSUM") as ptp,
            tc.tile_pool(name="dram", bufs=1, space="DRAM") as dp,
        ):
            w1 = cp.tile([128, HID], f32)
            w2 = cp.tile([128, HID], f32)
            wf = cp.tile([128, N_CLASSES], f32)
            bias = cp.tile([128, 3], f32)
            iota = cp.tile([128, 128], f32)
            mdst = cp.tile([128, NB], f32)
            mnrm = cp.tile([128, NB], f32)
            idxs = cp.tile([128, NB * 8], i16)
            ident = cp.tile([128, 128], f32)

            h0 = hp.tile([128, SH], f32, tag="h0")   # layer-l input, feature-major
            h1 = hp.tile([128, SH], f32, tag="h1")
            logits = sp.tile([128, TPC * N_CLASSES], f32, tag="lg")
            xs = sp.tile([128, TPC * N_CLASSES], f32, tag="xs")
            outn = sp.tile([128, TPC * N_CLASSES], f32, tag="on")
            sums = sp.tile([128, TPC], f32, tag="sm")
            lsum = sp.tile([128, TPC], f32, tag="ls")

            nc.sync.dma_start(w1[:], w1_d[:])
            nc.sync.dma_start(w2[:], w2_d[:])
            nc.sync.dma_start(wf[:], wf_d[:])
            nc.sync.dma_start(bias[:], bias_d[:])
            nc.sync.dma_start(iota[:], iota_d[:])
            nc.sync.dma_start(mdst[:], mdst_d[:])
            nc.sync.dma_start(mnrm[:], mnrm_d[:])
            nc.sync.dma_start(idxs[:], idx_d[:])
            nc.sync.dma_start(h0[:], xT_d[:])
            masks.make_identity(nc, ident[:])

            # per-tile block column offsets (shared host/device bookkeeping)
            boff = []
            acc = 0
            for t in range(TPC):
                boff.append(acc)
                acc += kA[t] + kB[t]

            hcur = h0
            hnxt = h1
            weights = (w1, w2, wf)
            for L in range(3):
                fo = FO[L]
                W = weights[L]

                # ---- dense phase: M shard = (H^T)^T @ W, node-major ----
                msh = dp.tile([SH, fo], f32, tag=f"msh{L}")
                mg = dp.tile([NPAD, fo], f32, tag=f"mg{L}")
                mstage = sp.tile([128, TPC * fo], f32, tag=f"mst{L % 2}")
                for t in range(TPC):
                    pm = pmp.tile([128, fo], f32, tag="pm")
                    nc.tensor.matmul(
                        pm[:], hcur[:, t * 128:(t + 1) * 128], W[:],
                        start=True, stop=True,
                    )
                    nc.scalar.activation(
                        mstage[:, t * fo:(t + 1) * fo], pm[:], Act.Copy
                    )
                nc.sync.dma_start(
                    msh[:].rearrange("(t p) f -> p t f", p=128),
                    mstage[:].rearrange("p (t f) -> p t f", f=fo),
                )

                # ---- AllGather M shards -> full M ----
                nc.gpsimd.collective_compute(
                    "AllGather",
                    mybir.AluOpType.bypass,
                    replica_groups=rg,
                    ins=[msh[:].opt()],
                    outs=[mg[:].opt()],
                )

                # ---- edge phase ----
                for t in range(TPC):
                    ka, kb = kA[t], kB[t]
                    kt = ka + kb
                    c0 = boff[t]
                    msgs = mp.tile([128, kt, fo], f32, tag="msgs")
                    if ka:
                        nc.gpsimd.dma_gather(
                            msgs[:, 0:ka, :],
                            mg[0:HALF, :],
                            idxs[:, c0 * 8:(c0 + ka) * 8],
                            ka * 128, ka * 128, fo,
                            queue_num=(2 * t) % 4,
                        )
                    if kb:
                        nc.gpsimd.dma_gather(
                            msgs[:, ka:kt, :],
                            mg[HALF:NPAD, :],
                            idxs[:, (c0 + ka) * 8:(c0 + kt) * 8],
                            kb * 128, kb * 128, fo,
                            queue_num=(2 * t + 1) % 4,
                        )
                    ph = php.tile([fo, 128], f32, tag="ph")
                    for b in range(kt):
                        g = c0 + b
                        oh = op_.tile([128, 128], f32, tag="oh")
                        nc.vector.tensor_scalar(
                            oh[:], iota[:],
                            mdst[:, g:g + 1], mnrm[:, g:g + 1],
                            Alu.is_equal, Alu.mult,
                        )
                        nc.tensor.matmul(
                            ph[:], msgs[:, b, :], oh[:],
                            start=(b == 0), stop=(b == kt - 1),
                        )
                    if L < 2:
                        nc.scalar.activation(
                            hnxt[:, t * 128:(t + 1) * 128], ph[:],
                            Act.Relu, bias=bias[:, L:L + 1],
                        )
                    else:
                        lt = lp.tile([N_CLASSES, 128], f32, tag="lt")
                        nc.scalar.activation(
                            lt[:], ph[:], Act.Identity,
                            bias=bias[:N_CLASSES, 2:3],
                        )
                        pt = ptp.tile([128, N_CLASSES], f32, tag="pt")
                        nc.tensor.transpose(
                            pt[:], lt[:], ident[:N_CLASSES, :N_CLASSES]
                        )
                        nc.vector.tensor_copy(
                            logits[:, t * N_CLASSES:(t + 1) * N_CLASSES], pt[:]
                        )
                hcur, hnxt = hnxt, hcur

            # ---- log-softmax over the 64 classes (free dim, node-major) ----
            lg3 = logits[:].rearrange("p (t c) -> p t c", c=N_CLASSES)
            xs3 = xs[:].rearrange("p (t c) -> p t c", c=N_CLASSES)
            on3 = outn[:].rearrange("p (t c) -> p t c", c=N_CLASSES)
            for t in range(TPC):
                mx = zp.tile([128, 1], f32, tag="mx")
                nc.vector.tensor_reduce(
                    mx[:], lg3[:, t, :], mybir.AxisListType.X, Alu.max
                )
                nc.vector.tensor_scalar(
                    xs3[:, t, :], lg3[:, t, :], mx[:], None, Alu.subtract
                )
                junk = zp.tile([128, N_CLASSES], f32, tag="jk")
                nc.scalar.activation(
                    junk[:], xs3[:, t, :], Act.Exp,
                    accum_out=sums[:, t:t + 1],
                )
            nc.scalar.activation(lsum[:], sums[:], Act.Ln)
            for t in range(TPC):
                nc.vector.tensor_scalar(
                    on3[:, t, :], xs3[:, t, :], lsum[:, t:t + 1], None,
                    Alu.subtract,
                )
            nc.sync.dma_start(
                out_d[:].rearrange("(t p) c -> p t c", p=128),
                on3,
            )

    nc.compile()
    return nc


def kernel(x, edge_index, W1, b1, W2, b2, Wf, bf):
    global LAST_RESULTS
    from concourse.bass_utils import run_bass_kernel_spmd

    x = np.asarray(x, dtype=np.float32)
    W1 = np.asarray(W1, dtype=np.float32)
    b1 = np.asarray(b1, dtype=np.float32)
    W2 = np.asarray(W2, dtype=np.float32)
    b2 = np.asarray(b2, dtype=np.float32)
    Wf = np.asarray(Wf, dtype=np.float32)
    bf = np.asarray(bf, dtype=np.float32)

    kA, kB, NB, idx_all, mdst_all, mnrm_all = _preprocess(x, edge_index)
    nc = _build(kA, kB, NB)

    xpad = np.zeros((NPAD, F_IN), np.float32)
    xpad[:N] = x
    bias = np.zeros((128, 3), np.float32)
    bias[:, 0] = b1
    bias[:, 1] = b2
    bias[:N_CLASSES, 2] = bf
    iota = np.tile(np.arange(128, dtype=np.float32), (128, 1))

    in_maps = []
    for c in range(NCORES):
        in_maps.append({
            "xT": np.ascontiguousarray(xpad[c * SH:(c + 1) * SH].T),
            "w1": W1, "w2": W2, "wf": Wf,
            "bias": bias, "iota": iota,
            "mdst": mdst_all[c], "mnrm": mnrm_all[c], "idx": idx_all[c],
        })

    res = run_bass_kernel_spmd(
        nc, in_maps, core_ids=list(range(NCORES)),
        trace=bool(os.environ.get("BASS_TRACE")),
    )
    LAST_RESULTS = res
    if res.exec_time_ns is not None:
        print(f"HW exec time: {res.exec_time_ns} ns")

    out = np.concatenate([r["logits"] for r in res.results], axis=0)
    return out[:N].astype(np.float32)


# revision 7
# speedup vs baseline: 1.9667x; 1.1477x over previous
"""3-layer GCN (DropGNN inference) on 8 Trainium2 NeuronCores.

Sharding: nodes row-sharded across 8 cores (6272 rows each, padded to 50176);
edges partitioned by destination node; weight matrices replicated; per-layer
transformed features M = H @ W are AllGathered so every core can gather
arbitrary source rows (equivalent to halo-exchanging gathered src features).

Per core, per layer:
  dense:  for each of 49 node tiles, PE computes M tile = (H^T tile)^T @ W
          (H kept feature-major in SBUF so no transposes are ever needed),
          ACT copies PSUM->SBUF, one DMA writes the [6272, fo] shard to DRAM.
  AG:     AllGather shards -> full M [50176, fo] in DRAM.
  edge:   for each dst tile: dma_gather pulls the tile's (padded) edge source
          rows (512B each) from M; DVE builds per-128-edge-block one-hot
          matrices onehot[e, d] = (iota[d] == dst_local[e]) * norm[e] in a
          single fused tensor_scalar op; PE accumulates
          PSUM[f, d] += msgs[e, f]^T @ onehot[e, d]
          over the tile's blocks -- segment_sum as dense matmuls. Epilogue:
          ACT relu(PSUM + b) -> next layer's feature-major H tile.
Final layer: PE-transpose logits to node-major, fused exp+accum log-softmax,
one DMA out. Host gathers the 8 shards and slices to 50000 rows.
"""

import os
import sys

import numpy as np

for _p in ("/opt/trn_rl_repo", "/opt/trn_rl_repo/concourse"):
    if _p not in sys.path:
        sys.path.insert(0, _p)

N = 50000
E = 625000
F_IN = 128
HID = 128
N_CLASSES = 64
NCORES = 8
TPC = 49                      # 128-node tiles per core
SH = TPC * 128                # 6272 nodes per core
NPAD = NCORES * SH            # 50176
HALF = NPAD // 2              # 25088 (< int16 max, gather index limit)
FO = (HID, HID, N_CLASSES)    # per-layer output widths

LAST_RESULTS = None           # BassKernelResults of the most recent run


def _preprocess(x, edge_index):
    """Sort/partition edges by destination, build per-core gather indices and
    per-block (dst_local, norm) metadata with fixed per-tile block counts."""
    ei = np.asarray(edge_index).astype(np.int64)
    loop = np.arange(N, dtype=np.int64)
    src = np.concatenate([ei[0], loop])
    dst = np.concatenate([ei[1], loop])

    deg = np.bincount(dst, minlength=NPAD).astype(np.float32)
    dinv = np.zeros(NPAD, np.float32)
    nz = deg > 0
    dinv[nz] = (np.float32(1.0) / np.sqrt(deg[nz])).astype(np.float32)
    norm = (dinv[src] * dinv[dst]).astype(np.float32)

    order = np.argsort(dst, kind="stable")
    src_s, dst_s, nrm_s = src[order], dst[order], norm[order]
    # global tile g = c*TPC + t covers dst rows [g*128, (g+1)*128)
    bounds = np.searchsorted(dst_s, np.arange(0, NPAD + 1, 128))

    ntiles = NCORES * TPC
    groups = []  # per (g, half): (src_rel, dst_local, norm)
    nA = np.zeros(ntiles, np.int64)
    nB = np.zeros(ntiles, np.int64)
    for g in range(ntiles):
        sl = slice(bounds[g], bounds[g + 1])
        es, el, en = src_s[sl], (dst_s[sl] - g * 128), nrm_s[sl]
        mA = es < HALF
        a = (es[mA], el[mA], en[mA])
        b = (es[~mA] - HALF, el[~mA], en[~mA])
        nA[g], nB[g] = len(a[0]), len(b[0])
        groups.append((a, b))

    nA2 = nA.reshape(NCORES, TPC)
    nB2 = nB.reshape(NCORES, TPC)
    kA = [int(x) for x in np.ceil(nA2.max(axis=0) / 128).astype(np.int64)]
    kB = [int(x) for x in np.ceil(nB2.max(axis=0) / 128).astype(np.int64)]
    for t in range(TPC):
        if kA[t] + kB[t] == 0:
            kA[t] = 1  # keep at least one (all-pad) block so PSUM gets started
    NB = sum(kA) + sum(kB)

    idx_all, mdst_all, mnrm_all = [], [], []
    for c in range(NCORES):
        idx16 = np.zeros((16, NB * 8), np.int16)
        mdst = np.full((128, NB), -1.0, np.float32)
        mnrm = np.zeros((128, NB), np.float32)
        bcol = 0
        for t in range(TPC):
            a, b = groups[c * TPC + t]
            for (es, el, en), k in ((a, kA[t]), (b, kB[t])):
                if k == 0:
                    continue
                n = len(es)
                if n:
                    s = np.arange(n)
                    mdst[s % 128, bcol + s // 128] = el.astype(np.float32)
                    mnrm[s % 128, bcol + s // 128] = en
                j = np.arange(k * 128)
                vals = np.zeros(k * 128, np.int16)
                vals[:n] = es.astype(np.int16)
                idx16[j % 16, bcol * 8 + j // 16] = vals
                bcol += k
        assert bcol == NB
        idx_all.append(np.tile(idx16, (8, 1)))
        mdst_all.append(mdst)
        mnrm_all.append(mnrm)

    return kA, kB, NB, idx_all, mdst_all, mnrm_all


def _build(kA, kB, NB):
    import concourse.bacc as bacc
    import concourse.mybir as mybir
    import concourse.tile as tile
    from concourse import masks

    f32 = mybir.dt.float32
    i16 = mybir.dt.int16
    Alu = mybir.AluOpType
    Act = mybir.ActivationFunctionType

    nc = bacc.Bacc(
        None, target_bir_lowering=False, num_devices=NCORES,
        num_swdge_queues=4,
    )

    xT_d = nc.dram_tensor("xT", [128, SH], f32, kind="ExternalInput")
    w1_d = nc.dram_tensor("w1", [128, HID], f32, kind="ExternalInput")
    w2_d = nc.dram_tensor("w2", [128, HID], f32, kind="ExternalInput")
    wf_d = nc.dram_tensor("wf", [128, N_CLASSES], f32, kind="ExternalInput")
    bias_d = nc.dram_tensor("bias", [128, 3], f32, kind="ExternalInput")
    iota_d = nc.dram_tensor("iota", [128, 128], f32, kind="ExternalInput")
    mdst_d = nc.dram_tensor("mdst", [128, NB], f32, kind="ExternalInput")
    mnrm_d = nc.dram_tensor("mnrm", [128, NB], f32, kind="ExternalInput")
    idx_d = nc.dram_tensor("idx", [128, NB * 8], i16, kind="ExternalInput")
    out_d = nc.dram_tensor("logits", [SH, N_CLASSES], f32, kind="ExternalOutput")

    rg = [list(range(NCORES))]

    with tile.TileContext(nc, num_cores=NCORES) as tc:
        with (
            tc.tile_pool(name="const", bufs=1) as cp,
            tc.tile_pool(name="hbuf", bufs=1) as hp,
            tc.tile_pool(name="stage", bufs=1) as sp,
            tc.tile_pool(name="msgs", bufs=4) as mp,
            tc.tile_pool(name="oh", bufs=8) as op_,
            tc.tile_pool(name="lt", bufs=4) as lp,
            tc.tile_pool(name="small", bufs=4) as zp,
            tc.tile_pool(name="pm", bufs=2, space="PSUM") as pmp,
            tc.tile_pool(name="ph", bufs=3, space="PSUM") as php,
            tc.tile_pool(name="pt", bufs=2, space="PSUM") as ptp,
            tc.tile_pool(name="dram", bufs=1, space="DRAM") as dp,
        ):
            w1 = cp.tile([128, HID], f32)
            w2 = cp.tile([128, HID], f32)
            wf = cp.tile([128, N_CLASSES], f32)
            bias = cp.tile([128, 3], f32)
            iota = cp.tile([128, 128], f32)
            mdst = cp.tile([128, NB], f32)
            mnrm = cp.tile([128, NB], f32)
            idxs = cp.tile([128, NB * 8], i16)
            ident = cp.tile([128, 128], f32)
            mdstn = cp.tile([128, NB], f32)   # -mdst (ACT one-hot path)
            mnrmn = cp.tile([128, NB], f32)   # -mnrm

            h0 = hp.tile([128, SH], f32, tag="h0")   # layer-l input, feature-major
            h1 = hp.tile([128, SH], f32, tag="h1")
            logits = sp.tile([128, TPC * N_CLASSES], f32, tag="lg")
            xs = sp.tile([128, TPC * N_CLASSES], f32, tag="xs")
            outn = sp.tile([128, TPC * N_CLASSES], f32, tag="on")
            sums = sp.tile([128, TPC], f32, tag="sm")
            lsum = sp.tile([128, TPC], f32, tag="ls")

            nc.sync.dma_start(w1[:], w1_d[:])
            nc.sync.dma_start(w2[:], w2_d[:])
            nc.sync.dma_start(wf[:], wf_d[:])
            nc.sync.dma_start(bias[:], bias_d[:])
            nc.sync.dma_start(iota[:], iota_d[:])
            nc.sync.dma_start(mdst[:], mdst_d[:])
            nc.sync.dma_start(mnrm[:], mnrm_d[:])
            nc.sync.dma_start(idxs[:], idx_d[:])
            nc.sync.dma_start(h0[:], xT_d[:])
            masks.make_identity(nc, ident[:])
            nc.vector.tensor_scalar_mul(mdstn[:], mdst[:], -1.0)
            nc.vector.tensor_scalar_mul(mnrmn[:], mnrm[:], -1.0)

            # per-tile block column offsets (shared host/device bookkeeping)
            boff = []
            acc = 0
            for t in range(TPC):
                boff.append(acc)
                acc += kA[t] + kB[t]

            hcur = h0
            hnxt = h1
            weights = (w1, w2, wf)
            for L in range(3):
                fo = FO[L]
                W = weights[L]

                # ---- dense phase: M shard = (H^T)^T @ W, node-major ----
                msh = dp.tile([SH, fo], f32, tag=f"msh{L}")
                mg = dp.tile([NPAD, fo], f32, tag=f"mg{L}")
                mstage = sp.tile([128, TPC * fo], f32, tag=f"mst{L % 2}")
                for t in range(TPC):
                    pm = pmp.tile([128, fo], f32, tag="pm")
                    nc.tensor.matmul(
                        pm[:], hcur[:, t * 128:(t + 1) * 128], W[:],
                        start=True, stop=True,
                    )
                    nc.scalar.activation(
                        mstage[:, t * fo:(t + 1) * fo], pm[:], Act.Copy
                    )
                nc.sync.dma_start(
                    msh[:].rearrange("(t p) f -> p t f", p=128),
                    mstage[:].rearrange("p (t f) -> p t f", f=fo),
                )

                # ---- AllGather M shards -> full M ----
                nc.gpsimd.collective_compute(
                    "AllGather",
                    mybir.AluOpType.bypass,
                    replica_groups=rg,
                    ins=[msh[:].opt()],
                    outs=[mg[:].opt()],
                )

                # ---- edge phase ----
                for t in range(TPC):
                    ka, kb = kA[t], kB[t]
                    kt = ka + kb
                    c0 = boff[t]
                    msgs = mp.tile([128, kt, fo], f32, tag="msgs")
                    if ka:
                        nc.gpsimd.dma_gather(
                            msgs[:, 0:ka, :],
                            mg[0:HALF, :],
                            idxs[:, c0 * 8:(c0 + ka) * 8],
                            ka * 128, ka * 128, fo,
                            queue_num=(2 * t) % 4,
                        )
                    if kb:
                        nc.gpsimd.dma_gather(
                            msgs[:, ka:kt, :],
                            mg[HALF:NPAD, :],
                            idxs[:, (c0 + ka) * 8:(c0 + kt) * 8],
                            kb * 128, kb * 128, fo,
                            queue_num=(2 * t + 1) % 4,
                        )
                    ph = php.tile([fo, 128], f32, tag="ph")
                    for b in range(kt):
                        g = c0 + b
                        oh = op_.tile([128, 128], f32, tag="oh")
                        if g % 3 == 0:
                            # ACT path: onehot = Relu(norm - norm*|iota - d|)
                            nc.scalar.activation(
                                oh[:], iota[:], Act.Abs,
                                bias=mdstn[:, g:g + 1],
                            )
                            nc.scalar.activation(
                                oh[:], oh[:], Act.Relu,
                                bias=mnrm[:, g:g + 1],
                                scale=mnrmn[:, g:g + 1],
                            )
                        else:
                            nc.vector.tensor_scalar(
                                oh[:], iota[:],
                                mdst[:, g:g + 1], mnrm[:, g:g + 1],
                                Alu.is_equal, Alu.mult,
                            )
                        nc.tensor.matmul(
                            ph[:], msgs[:, b, :], oh[:],
                            start=(b == 0), stop=(b == kt - 1),
                        )
                    if L < 2:
                        nc.scalar.activation(
                            hnxt[:, t * 128:(t + 1) * 128], ph[:],
                            Act.Relu, bias=bias[:, L:L + 1],
                        )
                    else:
                        lt = lp.tile([N_CLASSES, 128], f32, tag="lt")
                        nc.scalar.activation(
                            lt[:], ph[:], Act.Identity,
                            bias=bias[:N_CLASSES, 2:3],
                        )
                        pt = ptp.tile([128, N_CLASSES], f32, tag="pt")
                        nc.tensor.transpose(
                            pt[:], lt[:], ident[:N_CLASSES, :N_CLASSES]
                        )
                        nc.vector.tensor_copy(
                            logits[:, t * N_CLASSES:(t + 1) * N_CLASSES], pt[:]
                        )
                hcur, hnxt = hnxt, hcur

            # ---- log-softmax over the 64 classes (free dim, node-major) ----
            lg3 = logits[:].rearrange("p (t c) -> p t c", c=N_CLASSES)
            xs3 = xs[:].rearrange("p (t c) -> p t c", c=N_CLASSES)
            on3 = outn[:].rearrange("p (t c) -> p t c", c=N_CLASSES)
            for t in range(TPC):
                mx = zp.tile([128, 1], f32, tag="mx")
                nc.vector.tensor_reduce(
                    mx[:], lg3[:, t, :], mybir.AxisListType.X, Alu.max
                )
                nc.vector.tensor_scalar(
                    xs3[:, t, :], lg3[:, t, :], mx[:], None, Alu.subtract
                )
                junk = zp.tile([128, N_CLASSES], f32, tag="jk")
                nc.scalar.activation(
                    junk[:], xs3[:, t, :], Act.Exp,
                    accum_out=sums[:, t:t + 1],
                )
            nc.scalar.activation(lsum[:], sums[:], Act.Ln)
            for t in range(TPC):
                nc.vector.tensor_scalar(
                    on3[:, t, :], xs3[:, t, :], lsum[:, t:t + 1], None,
                    Alu.subtract,
                )
            nc.sync.dma_start(
                out_d[:].rearrange("(t p) c -> p t c", p=128),
                on3,
            )

    nc.compile()
    return nc


def kernel(x, edge_index, W1, b1, W2, b2, Wf, bf):
    global LAST_RESULTS
    from concourse.bass_utils import run_bass_kernel_spmd

    x = np.asarray(x, dtype=np.float32)
    W1 = np.asarray(W1, dtype=np.float32)
    b1 = np.asarray(b1, dtype=np.float32)
    W2 = np.asarray(W2, dtype=np.float32)
    b2 = np.asarray(b2, dtype=np.float32)
    Wf = np.asarray(Wf, dtype=np.float32)
    bf = np.asarray(bf, dtype=np.float32)

    kA, kB, NB, idx_all, mdst_all, mnrm_all = _preprocess(x, edge_index)
    nc = _build(kA, kB, NB)

    xpad = np.zeros((NPAD, F_IN), np.float32)
    xpad[:N] = x
    bias = np.zeros((128, 3), np.float32)
    bias[:, 0] = b1
    bias[:, 1] = b2
    bias[:N_CLASSES, 2] = bf
    iota = np.tile(np.arange(128, dtype=np.float32), (128, 1))

    in_maps = []
    for c in range(NCORES):
        in_maps.append({
            "xT": np.ascontiguousarray(xpad[c * SH:(c + 1) * SH].T),
            "w1": W1, "w2": W2, "wf": Wf,
            "bias": bias, "iota": iota,
            "mdst": mdst_all[c], "mnrm": mnrm_all[c], "idx": idx_all[c],
        })

    res = run_bass_kernel_spmd(
        nc, in_maps, core_ids=list(range(NCORES)),
        trace=bool(os.environ.get("BASS_TRACE")),
    )
    LAST_RESULTS = res
    if res.exec_time_ns is not None:
        print(f"HW exec time: {res.exec_time_ns} ns")

    out = np.concatenate([r["logits"] for r in res.results], axis=0)
    return out[:N].astype(np.float32)
